# revision 6
# baseline (speedup 1.0000x reference)
"""Trainium2 Bass kernel for nn_DecoderLayer (self-attn + cross-attn + FFN).

Sharding: 8 cores = 4 batch elements x 2 interleaved query-block sets
(core th takes query blocks {2i+th}, i=0..3, of 128 tokens), no
collectives.  Interleaving balances the causal triangle: local block qc
sees key blocks kc <= 2qc+1, so per-core score/AV work is the minimal
union and only the last two key blocks per qc need masking (applied
multiplicatively on DVE, not PE).

Precision: fp8e4 DoubleRow matmuls (2x PE rate) for q/k projections
(both attns), v2 + cross AV, and LN stat sums; fp8 weights pre-scaled
x16 host-side (exp scale absorbs 1/256 from q*k).  v1/self-AV stay
fp16 (early causal tokens average few keys so v/prob quantization
passes through un-averaged), as do Wo1/Wo2/FFN (residual-critical).
Attention output is produced transposed (va^T @ probs -> [dh, q]) so
softmax normalization is one reciprocal + one PE broadcast + one
multiply per head, with the denominator from an appended ones column.
"""

import sys

for _p in ("/opt/trn_rl_repo",):
    if _p not in sys.path:
        sys.path.insert(0, _p)

import numpy as np
import ml_dtypes

import concourse.bass as bass
from concourse import bacc
import concourse.mybir as mybir
import concourse.tile as tile

T, S, B, D, H, DH, DI = 1024, 1024, 4, 1024, 16, 64, 4096
P = 128
NQ = T // 2          # queries per core
DC = D // P          # 8  d chunks
DG = DC // 2         # 4  paired d chunks
KC = T // P          # 8  key chunks (self)
EC = S // P          # 8  key chunks (cross)
QC = NQ // P         # 4  query chunks
HC = (H * DH) // P   # 8  head-feature chunks
FIC = DI // P        # 32 ffn inner chunks
SCALE = 1.0 / (DH ** 0.5)
EPS = 1e-5
W16 = 16.0           # host prescale on fp8 weights
ESC = SCALE / (W16 * W16)   # exp scale compensating q,k both x16

F8 = mybir.dt.float8e4
F16 = mybir.dt.float16
F32 = mybir.dt.float32
BF16 = mybir.dt.bfloat16
AF = mybir.ActivationFunctionType
OP = mybir.AluOpType
DR = mybir.MatmulPerfMode.DoubleRow
E4 = ml_dtypes.float8_e4m3

_CACHE = {}


def _build(debug=False, ec_lim=EC, causal=True):
    nc = bacc.Bacc()

    dx8 = nc.dram_tensor("x8", [DG, P, 2, T], F8, kind="ExternalInput")
    dxq8 = nc.dram_tensor("xq8", [DG, P, 2, NQ], F8, kind="ExternalInput")
    dxq16 = nc.dram_tensor("xq16", [D, NQ], F16, kind="ExternalInput")
    denc8 = nc.dram_tensor("enc8", [DG, P, 2, S], F8, kind="ExternalInput")
    if causal:
        dmask = nc.dram_tensor("m16", [QC, P, 2, P], F16, kind="ExternalInput")
    else:
        dmask = nc.dram_tensor("m16f", [KC // 2, P, 2, NQ], F16,
                               kind="ExternalInput")
    demask = nc.dram_tensor("emask", [P, EC], F32, kind="ExternalInput")
    dw8 = {}
    for nm in ("wq1", "wk1", "wq2", "wk2", "wv2"):
        dw8[nm] = nc.dram_tensor(nm, [DG, P, 2, H * DH], F8, kind="ExternalInput")
    dw16 = {}
    for nm, shp in [("wv1", [D, H * DH]), ("wo1", [H * DH, D]),
                    ("wo2", [H * DH, D]), ("wff1", [D, DI]), ("wff2", [DI, D])]:
        dw16[nm] = nc.dram_tensor(nm, shp, F16, kind="ExternalInput")
    dln = {}
    for nm, w in [("g1", DC), ("b1", DC), ("g2", DC), ("b2", DC),
                  ("g3", DC), ("b3", DC), ("bf1", FIC), ("bf2", DC)]:
        dln[nm] = nc.dram_tensor(nm, [P, w], F32, kind="ExternalInput")
    dout = nc.dram_tensor("out_fm", [D, NQ], F32, kind="ExternalOutput")

    ECP = (ec_lim + 1) // 2   # cross kc pairs (last may be half)

    with tile.TileContext(nc) as tc:
        with tc.tile_pool(name="sb", bufs=1) as sb, \
             tc.tile_pool(name="pp", bufs=1, space="PSUM") as pp:

            # ---------------- constants ----------------
            ones8_ = sb.tile([P, 2, 16], F8, tag="ones8", name="ones8")
            nc.vector.memset(ones8_, 1.0)
            ones8 = ones8_[:, :, 0:1]   # pair stride 16: dual-fp8 LDW rule
            ones16 = sb.tile([1, 64], F16, tag="ones16", name="ones16")
            nc.vector.memset(ones16, 1.0)
            onep = sb.tile([1, P], F16, tag="onep", name="onep")
            nc.vector.memset(onep, 1.0)
            epsT = sb.tile([1, 1], F32, tag="epsT", name="epsT")
            nc.vector.memset(epsT, EPS)
            ln = {}
            for nm in dln:
                w = FIC if nm == "bf1" else DC
                ln[nm] = sb.tile([P, w], F32, tag=nm, name=nm)
                nc.sync.dma_start(ln[nm], dln[nm][:, :])
            emask_sb = sb.tile([P, EC], F32, tag="emask", name="emask")
            nc.sync.dma_start(emask_sb, demask[:, :])

            def fam(prefix, n, shape, dtype):
                return [sb.tile(shape, dtype, tag=f"{prefix}{i}", name=f"{prefix}{i}")
                        for i in range(n)]

            def dbg(name, tiles):
                if not debug:
                    return
                w = int(np.prod(tiles[0].shape[1:]))
                dt_ = nc.dram_tensor(name, [len(tiles) * P, w], tiles[0].dtype,
                                     kind="ExternalOutput")
                r = dt_.rearrange("(c p) t -> p c t", p=P)
                for i, t_ in enumerate(tiles):
                    if len(t_.shape) == 3:
                        t_ = t_.rearrange("p a b -> p (a b)")
                    elif len(t_.shape) == 4:
                        t_ = t_.rearrange("p a b c -> p (a b c)")
                    nc.sync.dma_start(r[:, i, :], t_)

            # ---------------- inputs ----------------
            x8_t = fam("x8", DG, [P, 2, T], F8)      # x, then c8 in-place, then enc8
            for g in range(DG):
                nc.sync.dma_start(x8_t[g], dx8[g])
            xq8_t = fam("xq8", DG, [P, 2, NQ], F8)   # later LN2/LN3 stat scratch
            for g in range(DG):
                nc.sync.dma_start(xq8_t[g], dxq8[g])
            xq16_t = fam("xq16", DC, [P, NQ], F16)
            dxq16_r = dxq16.rearrange("(dc p) t -> p dc t", p=P)
            for dc in range(DC):
                nc.sync.dma_start(xq16_t[dc], dxq16_r[:, dc, :])
            if causal:
                m16 = fam("m16", QC, [P, 2, P], F16)
                for qc in range(QC):
                    nc.sync.dma_start(m16[qc], dmask[qc])
            else:
                m16 = fam("m16", KC // 2, [P, 2, NQ], F16)
                for kcp in range(KC // 2):
                    nc.sync.dma_start(m16[kcp], dmask[kcp])

            wq8_t = fam("wq8", DG, [P, 2, H * DH], F8)   # wq1, later wq2
            wk8_t = fam("wk8", DG, [P, 2, H * DH], F8)   # wk1, wk2, wv2
            for g in range(DG):
                nc.sync.dma_start(wq8_t[g], dw8["wq1"][g])
                nc.sync.dma_start(wk8_t[g], dw8["wk1"][g])

            # ---------------- layernorm (feature-major) ----------------
            def ln_fm(src8, ntok, g, b, src32=None, out16=None, out32=None,
                      out8=None, cast8=False):
                """src8: DG paired fp8 tiles used for the stat matmuls (and as
                normalized source if src32 is None).  If cast8, fill src8 from
                src32 first.  Writes out16 (DC f16) / out32 (DC f32) /
                out8 (DG paired fp8)."""
                for j in range(ntok // 512):
                    js = slice(j * 512, (j + 1) * 512)
                    if cast8:
                        for dc in range(DC):
                            nc.vector.tensor_copy(
                                src8[dc // 2][:, dc % 2, js], src32[dc][:, js])
                    ms = pp.tile([1, 512], F32, tag="av", name="av", bufs=4)
                    qs = pp.tile([1, 512], F32, tag="av", name="av", bufs=4)
                    for gi in range(DG):
                        xs = src8[gi][:, :, js]
                        sq8 = sb.tile([P, 2, 512], F8, tag="lsq", name="lsq",
                                      bufs=2)
                        nc.vector.tensor_mul(sq8, xs, xs)
                        nc.tensor.matmul(ms, lhsT=ones8, rhs=xs,
                                         start=(gi == 0), stop=(gi == DG - 1),
                                         perf_mode=DR)
                        nc.tensor.matmul(qs, lhsT=ones8, rhs=sq8,
                                         start=(gi == 0), stop=(gi == DG - 1),
                                         perf_mode=DR)
                    def stt():
                        return sb.tile([1, 512], F32, tag="stat", name="stat",
                                       bufs=3)
                    mean = stt()
                    nc.vector.tensor_scalar_mul(mean, ms, 1.0 / D)
                    m2 = stt()
                    nc.vector.tensor_mul(m2, mean, mean)
                    var = stt()
                    nc.vector.tensor_scalar(var, qs, 1.0 / D, None, OP.mult)
                    nc.vector.tensor_sub(var, var, m2)
                    std = stt()
                    nc.scalar.activation(std, var, AF.Sqrt, bias=epsT)
                    rstd = stt()
                    nc.vector.reciprocal(rstd, std)
                    sh = sb.tile([1, 1024], F16, tag="sth", name="sth", bufs=1)
                    nc.vector.tensor_copy(sh[:, 0:512], mean)
                    nc.vector.tensor_copy(sh[:, 512:1024], rstd)
                    bcm = pp.tile([P, 512], F32, tag="big", name="big", bufs=4)
                    nc.tensor.matmul(bcm, lhsT=onep, rhs=sh[:, 0:512],
                                     start=True, stop=True)
                    bcr = pp.tile([P, 512], F32, tag="big", name="big", bufs=4)
                    nc.tensor.matmul(bcr, lhsT=onep, rhs=sh[:, 512:1024],
                                     start=True, stop=True)
                    for dc in range(DC):
                        srcd = (src32[dc][:, js] if src32 is not None
                                else src8[dc // 2][:, dc % 2, js])
                        t = sb.tile([P, 512], F16, tag="lnt", name="lnt", bufs=2)
                        nc.vector.tensor_sub(t, srcd, bcm)
                        nc.vector.tensor_mul(t, t, bcr)
                        gc, bc_ = g[:, dc:dc + 1], b[:, dc:dc + 1]
                        wrote = None
                        if out32 is not None:
                            nc.vector.tensor_scalar(
                                out32[dc][:, js], t, gc, bc_, OP.mult, OP.add)
                            wrote = out32[dc][:, js]
                        if out16 is not None:
                            if wrote is None:
                                nc.vector.tensor_scalar(
                                    out16[dc][:, js], t, gc, bc_, OP.mult, OP.add)
                                wrote = out16[dc][:, js]
                            else:
                                nc.vector.tensor_copy(out16[dc][:, js], wrote)
                        if out8 is not None:
                            o8 = out8[dc // 2][:, dc % 2, js]
                            if wrote is None:
                                nc.vector.tensor_scalar(
                                    o8, t, gc, bc_, OP.mult, OP.add)
                            else:
                                nc.vector.tensor_copy(o8, wrote)

            # ---------------- fp8 DoubleRow column projection ----------------
            def proj8(w8t, rhs8, n_fc, writer, rhs_w=NQ):
                for fc in range(n_fc):
                    fcs = slice(fc * P, (fc + 1) * P)
                    for j0 in range(0, rhs_w, 512):
                        w_ = min(512, rhs_w - j0)
                        js = slice(j0, j0 + w_)
                        acc = pp.tile([P, 512], F32, tag="big", name="big", bufs=4)
                        for g in range(DG):
                            nc.tensor.matmul(
                                acc[:, :w_], lhsT=w8t[g][:, :, fcs],
                                rhs=rhs8[g][:, :, js],
                                start=(g == 0), stop=(g == DG - 1), perf_mode=DR)
                        writer(fc, acc[:, :w_], js)

            # ---------------- fp16 column projection (streamed weights) ------
            def proj16(wd, rhs16, n_fc, writer):
                wr = wd.rearrange("(hc p) f -> p hc f", p=P)
                for fc in range(n_fc):
                    wt = sb.tile([P, HC, P], F16, tag="wo", name="wo", bufs=2)
                    nc.sync.dma_start(wt, wr[:, :, fc * P:(fc + 1) * P])
                    acc = pp.tile([P, 512], F32, tag="big", name="big", bufs=4)
                    for hc in range(HC):
                        nc.tensor.matmul(acc, lhsT=wt[:, hc, :], rhs=rhs16[hc],
                                         start=(hc == 0), stop=(hc == HC - 1))
                    writer(fc, acc)

            # ---------------- phase B: q1 (pre-LN so PE is busy early) ------
            q_t = fam("q", HC, [P, NQ], F16)         # q1, later q2
            proj8(wq8_t, xq8_t, HC,
                  lambda fc, acc, js: nc.vector.tensor_copy(q_t[fc][:, js], acc))

            # ---------------- LN1 ----------------
            c16_t = fam("c16", DC, [P, T], F16)
            c8_t = x8_t                              # in-place paired fp8
            ln_fm(x8_t, T, ln["g1"], ln["b1"], out16=c16_t, out8=c8_t)
            dbg("dbg_c", c16_t)

            # ---------------- k1 / v1 ----------------
            k_t = fam("k", HC, [P, T], F16)          # k1, later k2
            proj8(wk8_t, c8_t, HC,
                  lambda fc, acc, js: nc.vector.tensor_copy(k_t[fc][:, js], acc),
                  rhs_w=T)

            vb_t = fam("vb", KC, [P, H, DH + 1], F16)   # self V (+ones col)
            dwv1_r = dw16["wv1"].rearrange("(dc p) f -> p dc f", p=P)
            for grp in range(0, KC, 2):
                accs = [[pp.tile([P, 512], F32, tag="big", name="big", bufs=4)
                         for _ in range(2)] for _ in range(2)]
                for dc in range(DC):
                    for jn in range(2):
                        wvt = sb.tile([P, 512], F16, tag="wv", name="wv", bufs=2)
                        nc.sync.dma_start(
                            wvt, dwv1_r[:, dc, jn * 512:(jn + 1) * 512])
                        for i, tc8 in enumerate((grp, grp + 1)):
                            nc.tensor.matmul(
                                accs[i][jn],
                                lhsT=c16_t[dc][:, tc8 * P:(tc8 + 1) * P],
                                rhs=wvt,
                                start=(dc == 0), stop=(dc == DC - 1))
                for i, tc8 in enumerate((grp, grp + 1)):
                    for jn in range(2):
                        nc.vector.tensor_copy(
                            vb_t[tc8][:, jn * (H // 2):(jn + 1) * (H // 2), 0:DH],
                            accs[i][jn].rearrange("p (h d) -> p h d", h=H // 2))
                    nc.gpsimd.memset(vb_t[tc8][:, :, DH:DH + 1], 1.0)
            dbg("dbg_q1", q_t)
            dbg("dbg_k1", k_t)
            dbg("dbg_vb", vb_t)

            # ---------------- attention helpers ----------------
            def norm_av(av, vec, fch, hh):
                # av: psum [DH+1, 512]; divide rows 0:DH by row DH, write
                # vec[fch][64*hh : 64*hh+64, :] (fp16).
                rc = sb.tile([1, 512], F16, tag="rc", name="rc", bufs=2)
                with nc.allow_low_precision(
                        reason="softmax denom reciprocal; |den|>=1, fp16 ok"):
                    nc.vector.reciprocal(rc, av[DH:DH + 1, :])
                bc = pp.tile([DH, 512], F32, tag="av", name="av", bufs=4)
                nc.tensor.matmul(bc, lhsT=ones16, rhs=rc, start=True, stop=True)
                bcs = sb.tile([DH, 512], F16, tag="bcs", name="bcs", bufs=2)
                nc.vector.tensor_copy(bcs, bc)
                row = hh * DH
                nc.vector.tensor_mul(vec[fch][row:row + DH, :],
                                     av[0:DH, :], bcs)

            def attn_self(qt, kt, vb, vec):
                for fch in range(HC):
                    avs = [pp.tile([DH + 1, 512], F32, tag="av", name="av",
                                   bufs=4) for _ in range(2)]
                    for kcp in range(KC // 2):
                        q0 = kcp * P if causal else 0
                        pt = [sb.tile([P, 2, 512], F16, tag=f"pt{hh}",
                                      name=f"pt{hh}", bufs=2) for hh in range(2)]
                        for i in range(2):
                            kc = 2 * kcp + i
                            sps = []
                            for hh in range(2):
                                row = hh * DH
                                sp = pp.tile([P, 512], F32, tag="big",
                                             name="big", bufs=4)
                                nc.tensor.matmul(
                                    sp[:, q0:],
                                    lhsT=kt[fch][row:row + DH, kc * P:(kc + 1) * P],
                                    rhs=qt[fch][row:row + DH, q0:],
                                    start=True, stop=True,
                                    tile_position=(row, 0))
                                sps.append(sp)
                            for hh in range(2):
                                nc.scalar.activation(pt[hh][:, i, q0:],
                                                     sps[hh][:, q0:],
                                                     AF.Exp, scale=ESC)
                        # multiplicative causal mask on the diagonal block
                        for hh in range(2):
                            if causal:
                                dcol = slice(kcp * P, (kcp + 1) * P)
                                nc.vector.tensor_mul(pt[hh][:, :, dcol],
                                                     pt[hh][:, :, dcol],
                                                     m16[kcp])
                            else:
                                nc.vector.tensor_mul(pt[hh], pt[hh], m16[kcp])
                        for hh in range(2):
                            h = fch * 2 + hh
                            for i in range(2):
                                kc = 2 * kcp + i
                                nc.tensor.matmul(
                                    avs[hh][:, q0:],
                                    lhsT=vb[kc][:, h, :],
                                    rhs=pt[hh][:, i, q0:],
                                    start=(kc == 0), stop=(kc == KC - 1))
                    for hh in range(2):
                        norm_av(avs[hh], vec, fch, hh)

            def attn_cross(qt, kt, va, vec):
                for fch in range(HC):
                    avs = [pp.tile([DH + 1, 512], F32, tag="av", name="av",
                                   bufs=4) for _ in range(2)]
                    for kcp in range(ECP):
                        n_i = 2 if (2 * kcp + 1 < ec_lim) else 1
                        pt = [sb.tile([P, 2, 512], F8, tag=f"pc{hh}",
                                      name=f"pc{hh}", bufs=2) for hh in range(2)]
                        for i in range(n_i):
                            kc = 2 * kcp + i
                            sps = []
                            for hh in range(2):
                                row = hh * DH
                                sp = pp.tile([P, 512], F32, tag="big",
                                             name="big", bufs=4)
                                nc.tensor.matmul(
                                    sp,
                                    lhsT=kt[fch][row:row + DH, kc * P:(kc + 1) * P],
                                    rhs=qt[fch][row:row + DH, :],
                                    start=True, stop=True,
                                    tile_position=(row, 0))
                                sps.append(sp)
                            for hh in range(2):
                                nc.scalar.activation(
                                    pt[hh][:, i, :], sps[hh], AF.Exp,
                                    bias=emask_sb[:, kc:kc + 1], scale=ESC)
                        for hh in range(2):
                            h = fch * 2 + hh
                            if n_i == 2:
                                nc.tensor.matmul(
                                    avs[hh],
                                    lhsT=va[kcp][:, :, h, :],
                                    rhs=pt[hh],
                                    start=(kcp == 0), stop=(kcp == ECP - 1),
                                    perf_mode=DR)
                            else:
                                nc.tensor.matmul(
                                    avs[hh],
                                    lhsT=va[kcp][:, 0, h, :],
                                    rhs=pt[hh][:, 0, :],
                                    start=(kcp == 0), stop=(kcp == ECP - 1))
                    for hh in range(2):
                        norm_av(avs[hh], vec, fch, hh)

            # ---------------- self attention ----------------
            vec_t = fam("s", HC, [P, NQ], F16)       # vec1, vec2, later h3
            attn_self(q_t, k_t, vb_t, vec_t)
            dbg("dbg_vec", vec_t)

            # ---------------- Wo1 + residual ----------------
            r_t = fam("r", DC, [P, NQ], BF16)         # out1 -> h2 (f32)
            proj16(dw16["wo1"], vec_t, DC,
                   lambda fc, acc: nc.vector.tensor_add(r_t[fc], acc, xq16_t[fc]))
            dbg("dbg_out1", r_t)

            # ---------------- cross K/V (before LN2, keeps PE busy) ---------
            enc8_t = x8_t                            # reuse c8 slots
            for g in range(DG):
                nc.sync.dma_start(enc8_t[g], denc8[g])
                nc.sync.dma_start(wk8_t[g], dw8["wk2"][g])
                nc.sync.dma_start(wq8_t[g], dw8["wq2"][g])
            proj8(wk8_t, enc8_t, HC,
                  lambda fc, acc, js: nc.vector.tensor_copy(k_t[fc][:, js], acc),
                  rhs_w=ec_lim * P)
            for g in range(DG):
                nc.sync.dma_start(wk8_t[g], dw8["wv2"][g])

            va_t = fam("va", (EC + 1) // 2, [P, 2, H, DH + 1], F8)
            for tc8 in range(ec_lim):
                for jn in range(2):
                    acc = pp.tile([P, 512], F32, tag="big", name="big", bufs=4)
                    for g in range(DG):
                        nc.tensor.matmul(
                            acc,
                            lhsT=enc8_t[g][:, :, tc8 * P:(tc8 + 1) * P],
                            rhs=wk8_t[g][:, :, jn * 512:(jn + 1) * 512],
                            start=(g == 0), stop=(g == DG - 1), perf_mode=DR)
                    nc.vector.tensor_scalar_mul(
                        va_t[tc8 // 2][:, tc8 % 2,
                                       jn * (H // 2):(jn + 1) * (H // 2), 0:DH],
                        acc.rearrange("p (h d) -> p h d", h=H // 2), 1.0 / W16)
                nc.gpsimd.memset(va_t[tc8 // 2][:, tc8 % 2, :, DH:DH + 1], 1.0)

            # ---------------- LN2 + q2 ----------------
            h2h8_t = fam("h2h", DG, [P, 2, NQ], F8)
            ln_fm(xq8_t, NQ, ln["g2"], ln["b2"], src32=r_t, out32=r_t,
                  out8=h2h8_t, cast8=True)
            dbg("dbg_h2", r_t)
            proj8(wq8_t, h2h8_t, HC,
                  lambda fc, acc, js: nc.vector.tensor_copy(q_t[fc][:, js], acc))

            # ---------------- cross attention ----------------
            vec2_t = fam("s", HC, [P, NQ], F16)
            attn_cross(q_t, k_t, va_t, vec2_t)
            dbg("dbg_vec2", vec2_t)

            w_t = fam("w", DC, [P, NQ], BF16)         # out2
            proj16(dw16["wo2"], vec2_t, DC,
                   lambda fc, acc: nc.vector.tensor_add(w_t[fc], acc, r_t[fc]))
            dbg("dbg_out2", w_t)

            # ---------------- LN3 + FFN ----------------
            h3_t = fam("s", DC, [P, NQ], F16)
            ln_fm(xq8_t, NQ, ln["g3"], ln["b3"], src32=w_t, out16=h3_t,
                  cast8=True)
            dbg("dbg_h3", h3_t)

            g_t = fam("gg", FIC // 2, [P, NQ], F16)
            wf1r = dw16["wff1"].rearrange("(dc p) f -> p dc f", p=P)
            dout_r = dout.rearrange("(dc p) q -> p dc q", p=P)
            w2r = dw16["wff2"].rearrange("(fic p) f -> p fic f", p=P)
            NF = FIC // 2
            for half in range(2):
                for f in range(NF):
                    fc = half * NF + f
                    wt = sb.tile([P, DC, P], F16, tag="wf1", name="wf1", bufs=2)
                    nc.sync.dma_start(wt, wf1r[:, :, fc * P:(fc + 1) * P])
                    acc = pp.tile([P, 512], F32, tag="big", name="big", bufs=4)
                    for dc in range(DC):
                        nc.tensor.matmul(acc, lhsT=wt[:, dc, :], rhs=h3_t[dc],
                                         start=(dc == 0), stop=(dc == DC - 1))
                    nc.scalar.activation(g_t[f], acc, AF.Gelu,
                                         bias=ln["bf1"][:, fc:fc + 1], scale=1.0)
                for dc in range(DC):
                    w2t = sb.tile([P, NF, P], F16, tag="wf2", name="wf2",
                                  bufs=2)
                    nc.sync.dma_start(
                        w2t, w2r[:, half * NF:(half + 1) * NF,
                                 dc * P:(dc + 1) * P])
                    acc = pp.tile([P, 512], F32, tag="big", name="big", bufs=4)
                    for f in range(NF):
                        nc.tensor.matmul(acc, lhsT=w2t[:, f, :], rhs=g_t[f],
                                         start=(f == 0), stop=(f == NF - 1))
                    if half == 0:
                        nc.vector.tensor_copy(r_t[dc], acc)
                    else:
                        fin = sb.tile([P, NQ], F32, tag="fin", name="fin",
                                      bufs=2)
                        nc.vector.tensor_scalar_add(fin, acc,
                                                    ln["bf2"][:, dc:dc + 1])
                        nc.vector.tensor_add(fin, fin, r_t[dc])
                        nc.vector.tensor_add(fin, fin, w_t[dc])
                        nc.sync.dma_start(dout_r[:, dc, :], fin)

    nc.compile()
    return nc


def get_nc(debug=False, ec_lim=EC, causal=True):
    key = ("nc", debug, ec_lim, causal)
    if key not in _CACHE:
        _CACHE[key] = _build(debug=debug, ec_lim=ec_lim, causal=causal)
    return _CACHE[key]


def _pack8(a):
    """[D, N] f32 -> [DG, P, 2, N] fp8e4 (paired K chunks)."""
    n = a.shape[1]
    return np.ascontiguousarray(
        a.reshape(DG, 2, P, n).transpose(0, 2, 1, 3)).astype(E4)


def _qidx(th):
    """Global token indices for core query-half th (interleaved blocks)."""
    blk = (2 * np.arange(QC)[:, None] + th) * P + np.arange(P)[None, :]
    return blk.reshape(-1)


def make_in_maps(dec_inp, enc_out, dec_mask, enc_mask,
                 W_q1, W_kv1, W_o1, g1, b1,
                 W_q2, W_kv2, W_o2, g2, b2,
                 W_ff1, b_ff1, W_ff2, b_ff2, g3, b3,
                 causal=True):
    f16 = np.float16
    f32 = np.float32

    def colmajor(v, w):
        return np.ascontiguousarray(np.asarray(v, f32).reshape(w, P).T)

    wkv1 = np.asarray(W_kv1, f32)
    wkv2 = np.asarray(W_kv2, f32)
    shared = {
        "wq1": _pack8(np.asarray(W_q1, f32) * W16),
        "wk1": _pack8(wkv1[:, :H * DH] * W16),
        "wq2": _pack8(np.asarray(W_q2, f32) * W16),
        "wk2": _pack8(wkv2[:, :H * DH] * W16),
        "wv2": _pack8(wkv2[:, H * DH:] * W16),
        "wv1": np.ascontiguousarray(wkv1[:, H * DH:]).astype(f16),
        "wo1": np.asarray(W_o1, f16),
        "wo2": np.asarray(W_o2, f16),
        "wff1": np.asarray(W_ff1, f16),
        "wff2": np.asarray(W_ff2, f16),
        "g1": colmajor(g1, DC), "b1": colmajor(b1, DC),
        "g2": colmajor(g2, DC), "b2": colmajor(b2, DC),
        "g3": colmajor(g3, DC), "b3": colmajor(b3, DC),
        "bf1": colmajor(b_ff1, FIC), "bf2": colmajor(b_ff2, DC),
    }
    dec_inp = np.asarray(dec_inp, f32)
    enc_out = np.asarray(enc_out, f32)
    dec_mask = np.asarray(dec_mask)
    enc_mask = np.asarray(enc_mask)
    in_maps = []
    for core in range(8):
        b, th = divmod(core, 2)
        qi = _qidx(th)
        x = np.ascontiguousarray(dec_inp[:, b, :].T)        # [D, T]
        enc = np.ascontiguousarray(enc_out[:, b, :].T)      # [D, S]
        m = {}
        if causal:
            mm = np.empty((QC, P, 2, P), f16)
            for qc in range(QC):
                qg = (2 * qc + th) * P + np.arange(P)
                for i in range(2):
                    kc = 2 * qc + i
                    kg = kc * P + np.arange(P)
                    vis = ~dec_mask[np.ix_(qg, kg)][:, :, b]
                    mm[qc, :, i, :] = vis.T.astype(f16)
            m["m16"] = mm
        else:
            mm = np.empty((KC // 2, P, 2, NQ), f16)
            for kcp in range(KC // 2):
                for i in range(2):
                    kc = 2 * kcp + i
                    kg = kc * P + np.arange(P)
                    vis = ~dec_mask[np.ix_(qi, kg)][:, :, b]
                    mm[kcp, :, i, :] = vis.T.astype(f16)
            m["m16f"] = mm
        emask = np.ascontiguousarray(
            np.where(enc_mask[:, b], -10000.0, 0.0).astype(f32).reshape(EC, P).T)
        in_maps.append(dict(
            shared,
            x8=_pack8(x),
            xq8=_pack8(x[:, qi]),
            xq16=np.ascontiguousarray(x[:, qi]).astype(f16),
            enc8=_pack8(enc),
            emask=emask, **m))
    return in_maps


def assemble(results):
    out = np.empty((T, B, D), np.float32)
    for core in range(8):
        b, th = divmod(core, 2)
        out[_qidx(th), b, :] = results[core]["out_fm"].T
    return out


def derive_ec_lim(enc_mask):
    em = np.asarray(enc_mask)
    nvis = 0
    for b_ in range(em.shape[1]):
        col = em[:, b_]
        first = int(np.argmax(col)) if col.any() else S
        if col[:first].any() or not col[first:].all():
            return EC
        nvis = max(nvis, first)
    return max(1, min(EC, (nvis + P - 1) // P))


def causal_ok(dec_mask):
    dm = np.asarray(dec_mask)
    tri = np.triu(np.ones((T, T), bool), 1)
    return all(np.array_equal(dm[:, :, b_], tri) for b_ in range(dm.shape[2]))


def prepare(inputs):
    causal = causal_ok(inputs["dec_mask"])
    nc = get_nc(ec_lim=derive_ec_lim(inputs["enc_mask"]), causal=causal)
    return nc, make_in_maps(**inputs, causal=causal)


def kernel(**inputs):
    from concourse.bass_utils import run_bass_kernel_spmd

    nc, in_maps = prepare(inputs)
    res = run_bass_kernel_spmd(nc, in_maps, core_ids=list(range(8)))
    return assemble(res.results)


# revision 9
# speedup vs baseline: 1.0322x; 1.0322x over previous
"""Trainium2 Bass kernel for nn_DecoderLayer (self-attn + cross-attn + FFN).

Sharding: 8 cores = 4 batch elements x 2 interleaved query-block sets
(core th takes query blocks {2i+th}, i=0..3, of 128 tokens), no
collectives.  Interleaving balances the causal triangle: local block qc
sees key blocks kc <= 2qc+1, so per-core score/AV work is the minimal
union and only the last two key blocks per qc need masking (applied
multiplicatively on DVE, not PE).

Precision: fp8e4 DoubleRow matmuls (2x PE rate) for q/k projections
(both attns), v2 + cross AV, and LN stat sums; fp8 weights pre-scaled
x16 host-side (exp scale absorbs 1/256 from q*k).  v1/self-AV stay
fp16 (early causal tokens average few keys so v/prob quantization
passes through un-averaged), as do Wo1/Wo2/FFN (residual-critical).
Attention output is produced transposed (va^T @ probs -> [dh, q]) so
softmax normalization is one reciprocal + one PE broadcast + one
multiply per head, with the denominator from an appended ones column.
"""

import sys

for _p in ("/opt/trn_rl_repo",):
    if _p not in sys.path:
        sys.path.insert(0, _p)

import numpy as np
import ml_dtypes

import concourse.bass as bass
from concourse import bacc
import concourse.mybir as mybir
import concourse.tile as tile

T, S, B, D, H, DH, DI = 1024, 1024, 4, 1024, 16, 64, 4096
P = 128
NQ = T // 2          # queries per core
DC = D // P          # 8  d chunks
DG = DC // 2         # 4  paired d chunks
KC = T // P          # 8  key chunks (self)
EC = S // P          # 8  key chunks (cross)
QC = NQ // P         # 4  query chunks
HC = (H * DH) // P   # 8  head-feature chunks
FIC = DI // P        # 32 ffn inner chunks
SCALE = 1.0 / (DH ** 0.5)
EPS = 1e-5
W16 = 16.0           # host prescale on fp8 weights
ESC = SCALE / (W16 * W16)   # exp scale compensating q,k both x16

F8 = mybir.dt.float8e4
F16 = mybir.dt.float16
F32 = mybir.dt.float32
BF16 = mybir.dt.bfloat16
AF = mybir.ActivationFunctionType
OP = mybir.AluOpType
DR = mybir.MatmulPerfMode.DoubleRow
E4 = ml_dtypes.float8_e4m3

_CACHE = {}


def _build(debug=False, ec_lim=EC, causal=True):
    nc = bacc.Bacc()

    dx8 = nc.dram_tensor("x8", [DG, P, 2, T], F8, kind="ExternalInput")
    dxq8 = nc.dram_tensor("xq8", [DG, P, 2, NQ], F8, kind="ExternalInput")
    dxq16 = nc.dram_tensor("xq16", [D, NQ], F16, kind="ExternalInput")
    dx16h = nc.dram_tensor("x16h", [D, 2 * P], F16, kind="ExternalInput")
    denc8 = nc.dram_tensor("enc8", [DG, P, 2, S], F8, kind="ExternalInput")
    if causal:
        dmask = nc.dram_tensor("m16", [QC, P, 2, P], F16, kind="ExternalInput")
    else:
        dmask = nc.dram_tensor("m16f", [KC // 2, P, 2, NQ], F16,
                               kind="ExternalInput")
    demask = nc.dram_tensor("emask", [P, EC], F32, kind="ExternalInput")
    dw8 = {}
    for nm in ("wq1", "wk1", "wq2", "wk2", "wv2"):
        dw8[nm] = nc.dram_tensor(nm, [DG, P, 2, H * DH], F8, kind="ExternalInput")
    dw16 = {}
    for nm, shp in [("wv1", [D, H * DH]), ("wo1", [H * DH, D]),
                    ("wo2", [H * DH, D]), ("wff1", [D, DI]), ("wff2", [DI, D])]:
        dw16[nm] = nc.dram_tensor(nm, shp, F16, kind="ExternalInput")
    dln = {}
    for nm, w in [("g1", DC), ("b1", DC), ("g2", DC), ("b2", DC),
                  ("g3", DC), ("b3", DC), ("bf1", FIC), ("bf2", DC)]:
        dln[nm] = nc.dram_tensor(nm, [P, w], F32, kind="ExternalInput")
    dout = nc.dram_tensor("out_fm", [D, NQ], F32, kind="ExternalOutput")

    ECP = (ec_lim + 1) // 2   # cross kc pairs (last may be half)

    with tile.TileContext(nc) as tc:
        with tc.tile_pool(name="sb", bufs=1) as sb, \
             tc.tile_pool(name="pp", bufs=1, space="PSUM") as pp:

            # ---------------- constants ----------------
            ones8_ = sb.tile([P, 2, 16], F8, tag="ones8", name="ones8")
            nc.vector.memset(ones8_, 1.0)
            ones8 = ones8_[:, :, 0:1]   # pair stride 16: dual-fp8 LDW rule
            from concourse.masks import make_identity
            ident = sb.tile([P, P], F16, tag="ident", name="ident")
            make_identity(nc, ident)
            onep = sb.tile([1, P], F16, tag="onep", name="onep")
            nc.vector.memset(onep, 1.0)
            epsT = sb.tile([1, 1], F32, tag="epsT", name="epsT")
            nc.vector.memset(epsT, EPS)
            ln = {}
            for nm in dln:
                w = FIC if nm == "bf1" else DC
                ln[nm] = sb.tile([P, w], F32, tag=nm, name=nm)
                nc.sync.dma_start(ln[nm], dln[nm][:, :])
            emask_sb = sb.tile([P, EC], F32, tag="emask", name="emask")
            nc.sync.dma_start(emask_sb, demask[:, :])

            def fam(prefix, n, shape, dtype):
                return [sb.tile(shape, dtype, tag=f"{prefix}{i}", name=f"{prefix}{i}")
                        for i in range(n)]

            def dbg(name, tiles):
                if not debug:
                    return
                w = int(np.prod(tiles[0].shape[1:]))
                dt_ = nc.dram_tensor(name, [len(tiles) * P, w], tiles[0].dtype,
                                     kind="ExternalOutput")
                r = dt_.rearrange("(c p) t -> p c t", p=P)
                for i, t_ in enumerate(tiles):
                    if len(t_.shape) == 3:
                        t_ = t_.rearrange("p a b -> p (a b)")
                    elif len(t_.shape) == 4:
                        t_ = t_.rearrange("p a b c -> p (a b c)")
                    nc.sync.dma_start(r[:, i, :], t_)

            # ---------------- inputs ----------------
            x8_t = fam("x8", DG, [P, 2, T], F8)      # x, then c8 in-place, then enc8
            for g in range(DG):
                nc.sync.dma_start(x8_t[g], dx8[g])
            xq8_t = fam("xq8", DG, [P, 2, NQ], F8)   # later LN2/LN3 stat scratch
            for g in range(DG):
                nc.sync.dma_start(xq8_t[g], dxq8[g])
            x16h_t = fam("x16h", DC, [P, 2 * P], F16)
            dx16h_r = dx16h.rearrange("(dc p) t -> p dc t", p=P)
            for dc in range(DC):
                nc.sync.dma_start(x16h_t[dc], dx16h_r[:, dc, :])
            xq16_t = fam("xq16", DC, [P, NQ], F16)
            dxq16_r = dxq16.rearrange("(dc p) t -> p dc t", p=P)
            for dc in range(DC):
                nc.sync.dma_start(xq16_t[dc], dxq16_r[:, dc, :])
            if causal:
                m16 = fam("m16", QC, [P, 2, P], F16)
                for qc in range(QC):
                    nc.sync.dma_start(m16[qc], dmask[qc])
            else:
                m16 = fam("m16", KC // 2, [P, 2, NQ], F16)
                for kcp in range(KC // 2):
                    nc.sync.dma_start(m16[kcp], dmask[kcp])

            wq8_t = fam("wq8", DG, [P, 2, H * DH], F8)   # wq1, later wq2
            wk8_t = fam("wk8", DG, [P, 2, H * DH], F8)   # wk1, wk2, wv2
            for g in range(DG):
                nc.sync.dma_start(wq8_t[g], dw8["wq1"][g])
                nc.sync.dma_start(wk8_t[g], dw8["wk1"][g])

            # ---------------- layernorm (feature-major) ----------------
            def ln_fm(src8, ntok, g, b, src32=None, out16=None, out32=None,
                      out8=None, cast8=False, src16hi=None):
                """src8: DG paired fp8 tiles used for the stat matmuls (and as
                normalized source if src32 is None).  If cast8, fill src8 from
                src32 first.  Writes out16 (DC f16) / out32 (DC f32) /
                out8 (DG paired fp8)."""
                for j in range(ntok // 512):
                    js = slice(j * 512, (j + 1) * 512)
                    if cast8:
                        for dc in range(DC):
                            nc.vector.tensor_copy(
                                src8[dc // 2][:, dc % 2, js], src32[dc][:, js])
                    ms = pp.tile([1, 512], F32, tag="av", name="av", bufs=4)
                    qs = pp.tile([1, 512], F32, tag="av", name="av", bufs=4)
                    for gi in range(DG):
                        xs = src8[gi][:, :, js]
                        sq8 = sb.tile([P, 2, 512], F8, tag="lsq", name="lsq",
                                      bufs=2)
                        nc.vector.tensor_mul(sq8, xs, xs)
                        nc.tensor.matmul(ms, lhsT=ones8, rhs=xs,
                                         start=(gi == 0), stop=(gi == DG - 1),
                                         perf_mode=DR)
                        nc.tensor.matmul(qs, lhsT=ones8, rhs=sq8,
                                         start=(gi == 0), stop=(gi == DG - 1),
                                         perf_mode=DR)
                    def stt():
                        return sb.tile([1, 512], F32, tag="stat", name="stat",
                                       bufs=3)
                    mean = stt()
                    nc.vector.tensor_scalar_mul(mean, ms, 1.0 / D)
                    m2 = stt()
                    nc.vector.tensor_mul(m2, mean, mean)
                    var = stt()
                    nc.vector.tensor_scalar(var, qs, 1.0 / D, None, OP.mult)
                    nc.vector.tensor_sub(var, var, m2)
                    std = stt()
                    nc.scalar.activation(std, var, AF.Sqrt, bias=epsT)
                    rstd = stt()
                    nc.vector.reciprocal(rstd, std)
                    sh = sb.tile([1, 1024], F16, tag="sth", name="sth", bufs=1)
                    nc.vector.tensor_copy(sh[:, 0:512], mean)
                    nc.vector.tensor_copy(sh[:, 512:1024], rstd)
                    bcm = pp.tile([P, 512], F32, tag="big", name="big", bufs=4)
                    nc.tensor.matmul(bcm, lhsT=onep, rhs=sh[:, 0:512],
                                     start=True, stop=True)
                    bcr = pp.tile([P, 512], F32, tag="big", name="big", bufs=4)
                    nc.tensor.matmul(bcr, lhsT=onep, rhs=sh[:, 512:1024],
                                     start=True, stop=True)
                    for dc in range(DC):
                        srcd = (src32[dc][:, js] if src32 is not None
                                else src8[dc // 2][:, dc % 2, js])
                        t = sb.tile([P, 512], F16, tag="lnt", name="lnt", bufs=2)
                        nc.vector.tensor_sub(t, srcd, bcm)
                        nc.vector.tensor_mul(t, t, bcr)
                        gc, bc_ = g[:, dc:dc + 1], b[:, dc:dc + 1]
                        wrote = None
                        if out32 is not None:
                            nc.vector.tensor_scalar(
                                out32[dc][:, js], t, gc, bc_, OP.mult, OP.add)
                            wrote = out32[dc][:, js]
                        if out16 is not None:
                            if wrote is None:
                                nc.vector.tensor_scalar(
                                    out16[dc][:, js], t, gc, bc_, OP.mult, OP.add)
                                wrote = out16[dc][:, js]
                            else:
                                nc.vector.tensor_copy(out16[dc][:, js], wrote)
                        if out8 is not None:
                            o8 = out8[dc // 2][:, dc % 2, js]
                            if wrote is None:
                                nc.vector.tensor_scalar(
                                    o8, t, gc, bc_, OP.mult, OP.add)
                            else:
                                nc.vector.tensor_copy(o8, wrote)
                        if src16hi is not None and j == 0:
                            # redo first 256 cols from the fp16 source so v1
                            # of early (few-key) tokens avoids fp8 rounding
                            th_ = sb.tile([P, 2 * P], F16, tag="lnh",
                                          name="lnh", bufs=2)
                            nc.vector.tensor_sub(th_, src16hi[dc],
                                                 bcm[:, 0:2 * P])
                            nc.vector.tensor_mul(th_, th_, bcr[:, 0:2 * P])
                            nc.vector.tensor_scalar(
                                out16[dc][:, 0:2 * P], th_, gc, bc_,
                                OP.mult, OP.add)

            # ---------------- fp8 DoubleRow column projection ----------------
            def proj8(w8t, rhs8, n_fc, writer, rhs_w=NQ):
                for fc in range(n_fc):
                    fcs = slice(fc * P, (fc + 1) * P)
                    for j0 in range(0, rhs_w, 512):
                        w_ = min(512, rhs_w - j0)
                        js = slice(j0, j0 + w_)
                        acc = pp.tile([P, 512], F32, tag="big", name="big", bufs=4)
                        for g in range(DG):
                            nc.tensor.matmul(
                                acc[:, :w_], lhsT=w8t[g][:, :, fcs],
                                rhs=rhs8[g][:, :, js],
                                start=(g == 0), stop=(g == DG - 1), perf_mode=DR)
                        writer(fc, acc[:, :w_], js)

            # ---------------- fp16 column projection (streamed weights) ------
            def proj16(wd, rhs16, n_fc, writer):
                wr = wd.rearrange("(hc p) f -> p hc f", p=P)
                for fc in range(n_fc):
                    wt = sb.tile([P, HC, P], F16, tag="wo", name="wo", bufs=2)
                    nc.sync.dma_start(wt, wr[:, :, fc * P:(fc + 1) * P])
                    acc = pp.tile([P, 512], F32, tag="big", name="big", bufs=4)
                    for hc in range(HC):
                        nc.tensor.matmul(acc, lhsT=wt[:, hc, :], rhs=rhs16[hc],
                                         start=(hc == 0), stop=(hc == HC - 1))
                    writer(fc, acc)

            # ---------------- phase B: q1 (pre-LN so PE is busy early) ------
            q_t = fam("q", HC, [P, NQ], F16)         # q1, later q2
            proj8(wq8_t, xq8_t, HC,
                  lambda fc, acc, js: nc.vector.tensor_copy(q_t[fc][:, js], acc))

            # ---------------- LN1 ----------------
            c16_t = fam("c16", DC, [P, T], F16)
            c8_t = x8_t                              # in-place paired fp8
            ln_fm(x8_t, T, ln["g1"], ln["b1"], out16=c16_t, out8=c8_t,
                  src16hi=x16h_t)
            dbg("dbg_c", c16_t)

            # ---------------- k1 / v1 ----------------
            k_t = fam("k", HC, [P, T], F16)          # k1, later k2
            proj8(wk8_t, c8_t, HC,
                  lambda fc, acc, js: nc.vector.tensor_copy(k_t[fc][:, js], acc),
                  rhs_w=T)

            vb_t = fam("vb", KC, [P, H, DH + 1], F16)   # self V (+ones col)
            dwv1_r = dw16["wv1"].rearrange("(dc p) f -> p dc f", p=P)
            for grp in range(0, KC, 2):
                accs = [[pp.tile([P, 512], F32, tag="big", name="big", bufs=4)
                         for _ in range(2)] for _ in range(2)]
                for dc in range(DC):
                    for jn in range(2):
                        wvt = sb.tile([P, 512], F16, tag="wv", name="wv", bufs=2)
                        nc.sync.dma_start(
                            wvt, dwv1_r[:, dc, jn * 512:(jn + 1) * 512])
                        for i, tc8 in enumerate((grp, grp + 1)):
                            nc.tensor.matmul(
                                accs[i][jn],
                                lhsT=c16_t[dc][:, tc8 * P:(tc8 + 1) * P],
                                rhs=wvt,
                                start=(dc == 0), stop=(dc == DC - 1))
                for i, tc8 in enumerate((grp, grp + 1)):
                    for jn in range(2):
                        nc.vector.tensor_copy(
                            vb_t[tc8][:, jn * (H // 2):(jn + 1) * (H // 2), 0:DH],
                            accs[i][jn].rearrange("p (h d) -> p h d", h=H // 2))
                    nc.gpsimd.memset(vb_t[tc8][:, :, DH:DH + 1], 1.0)
            dbg("dbg_q1", q_t)
            dbg("dbg_k1", k_t)
            dbg("dbg_vb", vb_t)

            # ---------------- attention helpers ----------------
            def norm_av(avs, vec, fch):
                # avs[hh]: psum [P, 4, DH+1] (4 qc blocks, token-major);
                # divide cols 0:DH by col DH (per-partition scalar), then
                # transpose each [P, DH] block into vec[fch] (feature-major).
                vts = []
                for hh in range(2):
                    rc4 = sb.tile([P, 4], F32, tag="rc", name="rc", bufs=2)
                    nc.vector.reciprocal(rc4, avs[hh][:, :, DH])
                    vt4 = sb.tile([P, 4, DH], F16, tag=f"vt{hh}",
                                  name=f"vt{hh}", bufs=2)
                    for qc in range(QC):
                        nc.vector.tensor_scalar_mul(
                            vt4[:, qc, :], avs[hh][:, qc, 0:DH],
                            rc4[:, qc:qc + 1])
                    vts.append(vt4)
                for hh in range(2):
                    row = hh * DH
                    for qc in range(QC):
                        tp = pp.tile([DH, P], F16, tag="av", name="av", bufs=4)
                        nc.tensor.transpose(tp, vts[hh][:, qc, :], ident)
                        nc.vector.tensor_copy(
                            vec[fch][row:row + DH, qc * P:(qc + 1) * P], tp)

            def attn_self(qt, kt, vb, vec):
                for fch in range(HC):
                    avs = [pp.tile([P, 4, P], F32, tag="av", name="av",
                                   bufs=4) for _ in range(2)]
                    for kcp in range(KC // 2):
                        q0 = kcp * P if causal else 0
                        qc0 = kcp if causal else 0
                        pt = [sb.tile([P, 2, 512], F16, tag=f"pt{hh}",
                                      name=f"pt{hh}", bufs=2) for hh in range(2)]
                        for i in range(2):
                            kc = 2 * kcp + i
                            sps = []
                            for hh in range(2):
                                row = hh * DH
                                sp = pp.tile([P, 512], F32, tag="big",
                                             name="big", bufs=4)
                                nc.tensor.matmul(
                                    sp[:, q0:],
                                    lhsT=kt[fch][row:row + DH, kc * P:(kc + 1) * P],
                                    rhs=qt[fch][row:row + DH, q0:],
                                    start=True, stop=True,
                                    tile_position=(row, 0))
                                sps.append(sp)
                            for hh in range(2):
                                nc.scalar.activation(pt[hh][:, i, q0:],
                                                     sps[hh][:, q0:],
                                                     AF.Exp, scale=ESC)
                        # multiplicative causal mask on the diagonal block
                        for hh in range(2):
                            if causal:
                                dcol = slice(kcp * P, (kcp + 1) * P)
                                nc.vector.tensor_mul(pt[hh][:, :, dcol],
                                                     pt[hh][:, :, dcol],
                                                     m16[kcp])
                            else:
                                nc.vector.tensor_mul(pt[hh], pt[hh], m16[kcp])
                        for hh in range(2):
                            h = fch * 2 + hh
                            for i in range(2):
                                kc = 2 * kcp + i
                                for qc in range(qc0, QC):
                                    nc.tensor.matmul(
                                        avs[hh][:, qc, 0:DH + 1],
                                        lhsT=pt[hh][:, i, qc * P:(qc + 1) * P],
                                        rhs=vb[kc][:, h, :],
                                        start=(kc == 0 and qc == qc0),
                                        stop=(kcp == KC // 2 - 1 and i == 1
                                              and qc == QC - 1),
                                        skip_group_check=True)
                    norm_av(avs, vec, fch)

            def attn_cross(qt, kt, va, vec):
                for fch in range(HC):
                    avs = [pp.tile([P, 4, P], F32, tag="av", name="av",
                                   bufs=4) for _ in range(2)]
                    for kcp in range(ECP):
                        n_i = 2 if (2 * kcp + 1 < ec_lim) else 1
                        pt = [sb.tile([P, 2, 512], F8, tag=f"pc{hh}",
                                      name=f"pc{hh}", bufs=2) for hh in range(2)]
                        for i in range(n_i):
                            kc = 2 * kcp + i
                            sps = []
                            for hh in range(2):
                                row = hh * DH
                                sp = pp.tile([P, 512], F32, tag="big",
                                             name="big", bufs=4)
                                nc.tensor.matmul(
                                    sp,
                                    lhsT=kt[fch][row:row + DH, kc * P:(kc + 1) * P],
                                    rhs=qt[fch][row:row + DH, :],
                                    start=True, stop=True,
                                    tile_position=(row, 0))
                                sps.append(sp)
                            for hh in range(2):
                                nc.scalar.activation(
                                    pt[hh][:, i, :], sps[hh], AF.Exp,
                                    bias=emask_sb[:, kc:kc + 1], scale=ESC)
                        for hh in range(2):
                            h = fch * 2 + hh
                            for qc in range(QC):
                                qs = slice(qc * P, (qc + 1) * P)
                                st = (kcp == 0 and qc == 0)
                                sp_ = (kcp == ECP - 1 and qc == QC - 1)
                                if n_i == 2:
                                    nc.tensor.matmul(
                                        avs[hh][:, qc, 0:DH + 1],
                                        lhsT=pt[hh][:, :, qs],
                                        rhs=va[kcp][:, :, h, :],
                                        start=st, stop=sp_,
                                        skip_group_check=True,
                                        perf_mode=DR)
                                else:
                                    nc.tensor.matmul(
                                        avs[hh][:, qc, 0:DH + 1],
                                        lhsT=pt[hh][:, 0, qs],
                                        rhs=va[kcp][:, 0, h, :],
                                        start=st, stop=sp_,
                                        skip_group_check=True)
                    norm_av(avs, vec, fch)

            # ---------------- self attention ----------------
            vec_t = fam("s", HC, [P, NQ], F16)       # vec1, vec2, later h3
            attn_self(q_t, k_t, vb_t, vec_t)
            dbg("dbg_vec", vec_t)

            # ---------------- Wo1 + residual ----------------
            r_t = fam("r", DC, [P, NQ], BF16)         # out1 -> h2 (f32)
            proj16(dw16["wo1"], vec_t, DC,
                   lambda fc, acc: nc.vector.tensor_add(r_t[fc], acc, xq16_t[fc]))
            dbg("dbg_out1", r_t)

            # ---------------- cross K/V (before LN2, keeps PE busy) ---------
            enc8_t = x8_t                            # reuse c8 slots
            for g in range(DG):
                nc.sync.dma_start(enc8_t[g], denc8[g])
                nc.sync.dma_start(wk8_t[g], dw8["wk2"][g])
                nc.sync.dma_start(wq8_t[g], dw8["wq2"][g])
            proj8(wk8_t, enc8_t, HC,
                  lambda fc, acc, js: nc.vector.tensor_copy(k_t[fc][:, js], acc),
                  rhs_w=ec_lim * P)
            for g in range(DG):
                nc.sync.dma_start(wk8_t[g], dw8["wv2"][g])

            va_t = fam("va", (EC + 1) // 2, [P, 2, H, DH + 1], F8)
            for tc8 in range(ec_lim):
                for jn in range(2):
                    acc = pp.tile([P, 512], F32, tag="big", name="big", bufs=4)
                    for g in range(DG):
                        nc.tensor.matmul(
                            acc,
                            lhsT=enc8_t[g][:, :, tc8 * P:(tc8 + 1) * P],
                            rhs=wk8_t[g][:, :, jn * 512:(jn + 1) * 512],
                            start=(g == 0), stop=(g == DG - 1), perf_mode=DR)
                    nc.vector.tensor_scalar_mul(
                        va_t[tc8 // 2][:, tc8 % 2,
                                       jn * (H // 2):(jn + 1) * (H // 2), 0:DH],
                        acc.rearrange("p (h d) -> p h d", h=H // 2), 1.0 / W16)
                nc.gpsimd.memset(va_t[tc8 // 2][:, tc8 % 2, :, DH:DH + 1], 1.0)

            # ---------------- LN2 + q2 ----------------
            h2h8_t = fam("h2h", DG, [P, 2, NQ], F8)
            ln_fm(xq8_t, NQ, ln["g2"], ln["b2"], src32=r_t, out32=r_t,
                  out8=h2h8_t, cast8=True)
            dbg("dbg_h2", r_t)
            proj8(wq8_t, h2h8_t, HC,
                  lambda fc, acc, js: nc.vector.tensor_copy(q_t[fc][:, js], acc))

            # ---------------- cross attention ----------------
            vec2_t = fam("s", HC, [P, NQ], F16)
            attn_cross(q_t, k_t, va_t, vec2_t)
            dbg("dbg_vec2", vec2_t)

            w_t = fam("w", DC, [P, NQ], BF16)         # out2
            proj16(dw16["wo2"], vec2_t, DC,
                   lambda fc, acc: nc.vector.tensor_add(w_t[fc], acc, r_t[fc]))
            dbg("dbg_out2", w_t)

            # ---------------- LN3 + FFN ----------------
            h3_t = fam("s", DC, [P, NQ], F16)
            ln_fm(xq8_t, NQ, ln["g3"], ln["b3"], src32=w_t, out16=h3_t,
                  cast8=True)
            dbg("dbg_h3", h3_t)

            g_t = fam("gg", FIC // 2, [P, NQ], F16)
            wf1r = dw16["wff1"].rearrange("(dc p) f -> p dc f", p=P)
            dout_r = dout.rearrange("(dc p) q -> p dc q", p=P)
            w2r = dw16["wff2"].rearrange("(fic p) f -> p fic f", p=P)
            NF = FIC // 2
            for half in range(2):
                for f in range(NF):
                    fc = half * NF + f
                    wt = sb.tile([P, DC, P], F16, tag="wf1", name="wf1", bufs=2)
                    nc.sync.dma_start(wt, wf1r[:, :, fc * P:(fc + 1) * P])
                    acc = pp.tile([P, 512], F32, tag="big", name="big", bufs=4)
                    for dc in range(DC):
                        nc.tensor.matmul(acc, lhsT=wt[:, dc, :], rhs=h3_t[dc],
                                         start=(dc == 0), stop=(dc == DC - 1))
                    nc.scalar.activation(g_t[f], acc, AF.Gelu,
                                         bias=ln["bf1"][:, fc:fc + 1], scale=1.0)
                for dc in range(DC):
                    w2t = sb.tile([P, NF, P], F16, tag="wf2", name="wf2",
                                  bufs=2)
                    nc.sync.dma_start(
                        w2t, w2r[:, half * NF:(half + 1) * NF,
                                 dc * P:(dc + 1) * P])
                    acc = pp.tile([P, 512], F32, tag="big", name="big", bufs=4)
                    for f in range(NF):
                        nc.tensor.matmul(acc, lhsT=w2t[:, f, :], rhs=g_t[f],
                                         start=(f == 0), stop=(f == NF - 1))
                    if half == 0:
                        nc.vector.tensor_copy(r_t[dc], acc)
                    else:
                        fin = sb.tile([P, NQ], F32, tag="fin", name="fin",
                                      bufs=2)
                        nc.vector.tensor_scalar_add(fin, acc,
                                                    ln["bf2"][:, dc:dc + 1])
                        nc.vector.tensor_add(fin, fin, r_t[dc])
                        nc.vector.tensor_add(fin, fin, w_t[dc])
                        nc.sync.dma_start(dout_r[:, dc, :], fin)

    nc.compile()
    return nc


def get_nc(debug=False, ec_lim=EC, causal=True):
    key = ("nc", debug, ec_lim, causal)
    if key not in _CACHE:
        _CACHE[key] = _build(debug=debug, ec_lim=ec_lim, causal=causal)
    return _CACHE[key]


def _pack8(a):
    """[D, N] f32 -> [DG, P, 2, N] fp8e4 (paired K chunks)."""
    n = a.shape[1]
    return np.ascontiguousarray(
        a.reshape(DG, 2, P, n).transpose(0, 2, 1, 3)).astype(E4)


def _qidx(th):
    """Global token indices for core query-half th (interleaved blocks)."""
    blk = (2 * np.arange(QC)[:, None] + th) * P + np.arange(P)[None, :]
    return blk.reshape(-1)


def make_in_maps(dec_inp, enc_out, dec_mask, enc_mask,
                 W_q1, W_kv1, W_o1, g1, b1,
                 W_q2, W_kv2, W_o2, g2, b2,
                 W_ff1, b_ff1, W_ff2, b_ff2, g3, b3,
                 causal=True):
    f16 = np.float16
    f32 = np.float32

    def colmajor(v, w):
        return np.ascontiguousarray(np.asarray(v, f32).reshape(w, P).T)

    wkv1 = np.asarray(W_kv1, f32)
    wkv2 = np.asarray(W_kv2, f32)
    shared = {
        "wq1": _pack8(np.asarray(W_q1, f32) * W16),
        "wk1": _pack8(wkv1[:, :H * DH] * W16),
        "wq2": _pack8(np.asarray(W_q2, f32) * W16),
        "wk2": _pack8(wkv2[:, :H * DH] * W16),
        "wv2": _pack8(wkv2[:, H * DH:] * W16),
        "wv1": np.ascontiguousarray(wkv1[:, H * DH:]).astype(f16),
        "wo1": np.asarray(W_o1, f16),
        "wo2": np.asarray(W_o2, f16),
        "wff1": np.asarray(W_ff1, f16),
        "wff2": np.asarray(W_ff2, f16),
        "g1": colmajor(g1, DC), "b1": colmajor(b1, DC),
        "g2": colmajor(g2, DC), "b2": colmajor(b2, DC),
        "g3": colmajor(g3, DC), "b3": colmajor(b3, DC),
        "bf1": colmajor(b_ff1, FIC), "bf2": colmajor(b_ff2, DC),
    }
    dec_inp = np.asarray(dec_inp, f32)
    enc_out = np.asarray(enc_out, f32)
    dec_mask = np.asarray(dec_mask)
    enc_mask = np.asarray(enc_mask)
    in_maps = []
    for core in range(8):
        b, th = divmod(core, 2)
        qi = _qidx(th)
        x = np.ascontiguousarray(dec_inp[:, b, :].T)        # [D, T]
        enc = np.ascontiguousarray(enc_out[:, b, :].T)      # [D, S]
        m = {}
        if causal:
            mm = np.empty((QC, P, 2, P), f16)
            for qc in range(QC):
                qg = (2 * qc + th) * P + np.arange(P)
                for i in range(2):
                    kc = 2 * qc + i
                    kg = kc * P + np.arange(P)
                    vis = ~dec_mask[np.ix_(qg, kg)][:, :, b]
                    mm[qc, :, i, :] = vis.T.astype(f16)
            m["m16"] = mm
        else:
            mm = np.empty((KC // 2, P, 2, NQ), f16)
            for kcp in range(KC // 2):
                for i in range(2):
                    kc = 2 * kcp + i
                    kg = kc * P + np.arange(P)
                    vis = ~dec_mask[np.ix_(qi, kg)][:, :, b]
                    mm[kcp, :, i, :] = vis.T.astype(f16)
            m["m16f"] = mm
        emask = np.ascontiguousarray(
            np.where(enc_mask[:, b], -10000.0, 0.0).astype(f32).reshape(EC, P).T)
        in_maps.append(dict(
            shared,
            x8=_pack8(x),
            x16h=np.ascontiguousarray(x[:, 0:2 * P]).astype(f16),
            xq8=_pack8(x[:, qi]),
            xq16=np.ascontiguousarray(x[:, qi]).astype(f16),
            enc8=_pack8(enc),
            emask=emask, **m))
    return in_maps


def assemble(results):
    out = np.empty((T, B, D), np.float32)
    for core in range(8):
        b, th = divmod(core, 2)
        out[_qidx(th), b, :] = results[core]["out_fm"].T
    return out


def derive_ec_lim(enc_mask):
    em = np.asarray(enc_mask)
    nvis = 0
    for b_ in range(em.shape[1]):
        col = em[:, b_]
        first = int(np.argmax(col)) if col.any() else S
        if col[:first].any() or not col[first:].all():
            return EC
        nvis = max(nvis, first)
    return max(1, min(EC, (nvis + P - 1) // P))


def causal_ok(dec_mask):
    dm = np.asarray(dec_mask)
    tri = np.triu(np.ones((T, T), bool), 1)
    return all(np.array_equal(dm[:, :, b_], tri) for b_ in range(dm.shape[2]))


def prepare(inputs):
    causal = causal_ok(inputs["dec_mask"])
    nc = get_nc(ec_lim=derive_ec_lim(inputs["enc_mask"]), causal=causal)
    return nc, make_in_maps(**inputs, causal=causal)


def kernel(**inputs):
    from concourse.bass_utils import run_bass_kernel_spmd

    nc, in_maps = prepare(inputs)
    res = run_bass_kernel_spmd(nc, in_maps, core_ids=list(range(8)))
    return assemble(res.results)
